# revision 37
# baseline (speedup 1.0000x reference)
"""Converged inhibition (FFT deconvolution) as a circulant matmul on TRN2.

reference:  out = Re(ifft(fft(a, axis=-1) / fft(delta - w)))
identity :  out = a (circular-conv) g,  g = Re(ifft(1 / fft(delta - w)))
         => out[r, n] = sum_m a[r, m] * H[m, n],  H[m, n] = g[(n - m) mod 63]

So the whole op is X[524288, 63] @ H[63, 63].  Data-parallel over 8 cores
(65536 rows each).  Per core (cost-model timeline ~99 us; DMA busy floor
~92 us, PE ~90 us — the ridge):

  DMA in  [128, 1008] f32 (4 KB/partition, contiguous HBM)       (sync/HWDGE)
  PE      transpose row-pairs  [128,126] -> [126,128] psum       (4 per bank)
  DVE     copy  psum_t [126,512] -> sbuf                         (1 per group)
  PE      matmul lhsT=Xt[126,128], rhs=blockdiag(H,H)[126,126]
          -> out [128,126] psum  == natural row-pair layout      (4 per bank)
  ACT     copy  psum_o -> back into the io SBUF tile (in place)  (1 per group)
  DMA out [128, 1008]                                            (scalar/HWDGE)

The io SBUF buffer is fully resident (32 slots, 129 KB/partition): loads
never reuse a slot, and outputs overwrite the consumed inputs in place, so
no DMA ever needs a slot-release wait.  This matters because this walrus
build rejects instructions with >2 sync commands (waits+incs): every
engine instruction here carries at most 1 wait.  Tiny "absorber" ops (din/
dxt/po-opener on PE, tr on ACT) each consume one cross-engine wait ahead
of the real work, `_strip_redundant_waits` removes provably-satisfied
same-engine waits, and `_patch_drain` splits the kernel-tail drain's 11
waits across a chain of drains.
"""

import numpy as np

SCOPE = 63
N_CORES = 8
ROWS_PER_CORE = 64 * 64 * 128 // N_CORES  # 65536
NL = 32                    # DMA loads per core
LOAD_F = 1008              # f32 per partition per load (8 rows of 63)
GROUPS_PER_LOAD = 2        # 504 f32 each
PAIR = 126                 # one double-row
TILES_PER_GROUP = 4        # 4 double-tiles = 1 psum bank

_CACHE = {}


def _inverse_kernel(inhibition_filter: np.ndarray) -> np.ndarray:
    """g = Re(ifft(1 / fft(delta - w))) computed in float64 on host."""
    k = -inhibition_filter.astype(np.float64)
    k[0] += 1.0
    return np.real(np.fft.ifft(1.0 / np.fft.fft(k)))


def _build_consts(inhibition_filter: np.ndarray):
    g = _inverse_kernel(inhibition_filter)
    idx = (np.arange(SCOPE)[None, :] - np.arange(SCOPE)[:, None]) % SCOPE
    H = g[idx].astype(np.float32)          # [m, n]
    H2 = np.zeros((2 * SCOPE, 2 * SCOPE), np.float32)
    H2[:SCOPE, :SCOPE] = H
    H2[SCOPE:, SCOPE:] = H
    ident = np.eye(128, dtype=np.float32)
    return H2, ident


def _strip_redundant_waits(nc):
    """Remove same-engine semaphore waits that are guaranteed satisfied.

    Engines execute their (non-DMA) instructions in order and increment
    their engine semaphore at completion.  A wait `sem_E >= v` on the k-th
    instruction of engine E is therefore always satisfied when v <= the
    cumulative increments from E's earlier non-DMA instructions.  Walrus
    rejects instructions with too many sync commands, and Tile emits these
    provably-redundant waits (its minimality is per-proc, not transitive).
    DMA completions are asynchronous; their sems are never inc'd by engine
    instruction retirement, so they are untouched.
    """
    insts = list(nc.all_instructions())
    # sem id -> set of (engine, is_dma) that update it
    updaters = {}
    for i in insts:
        eng = i.engine
        si = i.sync_info
        if si is None:
            continue
        is_dma = type(i).__name__ in ("InstDMACopy", "InstDmaTransposeAnt",
                                      "InstTensorLoad", "InstTensorSave",
                                      "InstCollectiveCompute")
        for u in (si.on_update or []):
            if u.update_mode == "sem-inc":
                updaters.setdefault(u.id, set()).add((eng, is_dma))

    def engine_private(sem_id, eng):
        ups = updaters.get(sem_id, set())
        return ups and all(e == eng and not d for (e, d) in ups)

    counts = {}
    removed = 0
    for i in insts:
        eng = i.engine
        si = i.sync_info
        if si is None:
            continue
        waits = si.on_wait or []
        if waits:
            keep = []
            for w in waits:
                if (w.wait_mode == "sem-ge-imm"
                        and engine_private(w.id, eng)
                        and w.wait_value <= counts.get((eng, w.id), 0)):
                    removed += 1
                else:
                    keep.append(w)
            if len(keep) != len(waits):
                si.on_wait = keep
        is_dma = type(i).__name__ in ("InstDMACopy", "InstDmaTransposeAnt",
                                      "InstTensorLoad", "InstTensorSave",
                                      "InstCollectiveCompute")
        if not is_dma:
            for u in (si.on_update or []):
                if u.update_mode == "sem-inc":
                    key = (eng, u.id)
                    counts[key] = counts.get(key, 0) + u.update_value
    return removed


def _patch_drain(tile_mod):
    """Split the kernel-tail drain's sem waits across a chain of drains.

    Tile emits one Drain instruction waiting on every proc (11 sems); this
    walrus build rejects instructions with that many sync commands.  SP
    executes the chain serially, so N drains with <=1 wait each are
    equivalent.
    """
    if getattr(tile_mod.TileContext, "_drain_split_patched", False):
        return
    from concourse.vector_clock import ScopedClock

    def _drain_and_barrier(self, tick_clock, wait_clock):
        nc = self.nc
        drain_inst = nc.sync.drain()
        wait_clock.add_sem_waits(
            drain_inst.ins, ScopedClock({None: tick_clock.global_clock})
        )
        si = drain_inst.ins.sync_info
        waits = list((si.on_wait if si else None) or [])
        if len(waits) > 1:
            si.on_wait = waits[:1]
            for w in waits[1:]:
                d2 = nc.sync.drain()
                si2 = d2.ins.sync_info
                if si2 is None:
                    wait_clock.add_sem_waits(
                        d2.ins, ScopedClock({None: tick_clock.global_clock})
                    )
                    si2 = d2.ins.sync_info
                si2.on_wait = [w]
        nc.all_engine_barrier()
        assert self.sems is not None
        popped = nc._tile_sem_poison_stack.pop()
        assert popped is self._sem_poison
        nc.clear_and_free_semaphores(list(self.sems.allocated().values()))
        nc.all_engine_barrier()

    tile_mod.TileContext._drain_and_barrier = _drain_and_barrier
    tile_mod.TileContext._drain_split_patched = True


def _build_program(nl: int, in_bufs: int = 3, st_bufs: int = 3,
                   split_load: int = 1, alt_load_engine: bool = False,
                   const_eng: str = "gpsimd", load_eng: str = "sync",
                   store_eng: str = "scalar", strip: bool = True,
                   pt_bufs: int = 3, po_bufs: int = 4, xt_bufs: int = 8,
                   warm_spin: int = 0, t_f32r: bool = False):
    import concourse.bass as bass
    import concourse.mybir as mybir
    from concourse import tile

    _patch_drain(tile)

    f32 = mybir.dt.float32
    f32r = mybir.dt.float32r
    nc = bass.Bass("TRN2", target_bir_lowering=False, debug=False,
                   num_devices=N_CORES)

    act = nc.dram_tensor("activations", [nl, 128, LOAD_F], f32,
                         kind="ExternalInput").ap()
    h2d = nc.dram_tensor("h2", [PAIR, PAIR], f32, kind="ExternalInput").ap()
    idd = nc.dram_tensor("ident", [128, 128], f32, kind="ExternalInput").ap()
    out = nc.dram_tensor("out", [nl, 128, LOAD_F], f32,
                         kind="ExternalOutput").ap()

    with tile.TileContext(nc) as tc:
        with (
            tc.tile_pool(name="consts", bufs=1) as consts,
            tc.tile_pool(name="inp", bufs=nl) as inp,
            tc.tile_pool(name="xt", bufs=xt_bufs) as xtp,
            tc.tile_pool(name="trash", bufs=1) as trp,
            tc.tile_pool(name="psum_t", bufs=pt_bufs, space="PSUM") as pst,
            tc.tile_pool(name="psum_o", bufs=po_bufs, space="PSUM") as pso,
            tc.tile_pool(name="scratch", bufs=1, space="PSUM") as scratch,
        ):
            eng = {"sync": nc.sync, "scalar": nc.scalar, "gpsimd": nc.gpsimd}
            h2_sb = consts.tile([PAIR, PAIR], f32)
            id_sb = consts.tile([128, 128], f32)
            eng[const_eng].dma_start(out=h2_sb[:], in_=h2d)
            eng[const_eng].dma_start(out=id_sb[:], in_=idd)

            if warm_spin:
                # Spin the PE on a zeroed tile while the first load is in
                # flight: ramps the HAM clock gate / pstate before real work.
                zt = consts.tile([128, 128], f32, tag="warmz")
                nc.gpsimd.memset(zt[:], 0.0)
                for w in range(warm_spin):
                    ztp = scratch.tile([PAIR, 128], f32, tag="din")
                    nc.tensor.matmul(ztp[:], zt[:, 0:PAIR], zt[:],
                                     is_transpose=True)
            # Warm-up dummies: consume the const-load waits on PE once.
            wt = scratch.tile([PAIR, 128], f32, tag="din")
            nc.tensor.matmul(wt[:], id_sb[:, 0:PAIR], id_sb[:],
                             is_transpose=True)
            wo = scratch.tile([128, PAIR], f32, tag="din")
            nc.tensor.matmul(wo[:], id_sb[0:PAIR, :], h2_sb[:])

            for L in range(nl):
                # Full-residency i/o buffer: every load gets a fresh SBUF
                # slot (no WAW/release waits on the DMA) and the outputs are
                # written back in place, so the store only waits on copy2.
                in_t = inp.tile([128, LOAD_F], f32)
                ld_eng = nc.gpsimd if (alt_load_engine and L % 2) else eng[load_eng]
                ld_eng.dma_start(out=in_t[:], in_=act[L])
                # ACT wait-absorber: observe the load completion on ACT so
                # the in-place copy2 below only waits on PE.
                tr = trp.tile([1, 1], f32, tag="tr")
                nc.scalar.copy(tr[:], in_t[0:1, 0:1])
                for grp in range(GROUPS_PER_LOAD):
                    base = 504 * grp
                    # PE wait-absorber: a 1x1 matmul reading in_t consumes the
                    # load-DMA-done wait, so the first transpose below only
                    # waits on its psum slot release.
                    din = scratch.tile([1, 1], f32, tag="din")
                    nc.tensor.matmul(din[:], in_t[0:1, base:base + 1],
                                     id_sb[0:1, 0:1])
                    pt = pst.tile([PAIR, 512], f32)
                    for k in range(TILES_PER_GROUP):
                        off = base + PAIR * k
                        # Note: t_f32r stays False — the BIR verifier
                        # requires fp32r inputs pre-rounded (it is a reduced
                        # precision format), so it would truncate the data.
                        cast = (lambda ap: ap.bitcast(f32r)) if t_f32r else (lambda ap: ap)
                        nc.tensor.matmul(
                            cast(pt[:, 128 * k:128 * (k + 1)]),
                            cast(in_t[:, off:off + PAIR]),
                            cast(id_sb[:]),
                            is_transpose=True,
                            start=(k == 0),
                            stop=(k == TILES_PER_GROUP - 1),
                        )
                    xt = xtp.tile([PAIR, 512], f32)
                    nc.vector.tensor_copy(xt[:], pt[:])
                    po = pso.tile([128, 512], f32)
                    # PE wait-absorber for the psum_o slot release (ACT): the
                    # group opener writes spare column 504 (never read), so
                    # the real matmuls below carry no ACT wait.
                    nc.tensor.matmul(po[:, 504:505], id_sb[0:1, :],
                                     id_sb[0:1, 0:1], start=True, stop=False)
                    # PE wait-absorber: consumes the xt-ready (DVE) wait.
                    dxt = scratch.tile([1, 1], f32, tag="din")
                    nc.tensor.matmul(dxt[:], xt[0:1, 0:1], id_sb[0:1, 0:1])
                    for k in range(TILES_PER_GROUP):
                        nc.tensor.matmul(
                            po[:, PAIR * k:PAIR * (k + 1)],
                            xt[:, 128 * k:128 * (k + 1)],
                            h2_sb[:],
                            start=False,
                            stop=(k == TILES_PER_GROUP - 1),
                        )
                    nc.scalar.copy(in_t[:, base:base + 504], po[:, 0:504])
                eng[store_eng].dma_start(out=out[L], in_=in_t[:])

    if strip:
        # Safe on hardware (engines retire instructions in order); the
        # CoreSim race detector, however, only understands semaphores, so
        # sim runs should build with strip=False.
        _strip_redundant_waits(nc)
    return nc


def _get_program(nl: int):
    if nl not in _CACHE:
        _CACHE[nl] = _build_program(nl)
    return _CACHE[nl]


def kernel(activations: np.ndarray, inhibition_filter: np.ndarray,
           trace: bool = False):
    from concourse.bass_utils import run_bass_kernel_spmd

    activations = np.ascontiguousarray(activations, dtype=np.float32)
    H2, ident = _build_consts(np.asarray(inhibition_filter, np.float32))

    shards = activations.reshape(N_CORES, NL, 128, LOAD_F)
    in_maps = [
        {"activations": shards[c], "h2": H2, "ident": ident}
        for c in range(N_CORES)
    ]
    nc = _get_program(NL)
    try:
        res = run_bass_kernel_spmd(nc, in_maps, list(range(N_CORES)),
                                   trace=trace)
    except ModuleNotFoundError:
        # NTFF profile hook unavailable in this axon build
        res = run_bass_kernel_spmd(nc, in_maps, list(range(N_CORES)),
                                   trace=False)
    full = np.concatenate([res.results[c]["out"].reshape(1, -1)
                           for c in range(N_CORES)], axis=0)
    out = full.reshape(64, 64, 128, SCOPE)
    if trace:
        kernel.last_results = res
    return out


# revision 38
# speedup vs baseline: 143.9890x; 143.9890x over previous
"""Converged inhibition (FFT deconvolution) as a circulant matmul on TRN2.

reference:  out = Re(ifft(fft(a, axis=-1) / fft(delta - w)))
identity :  out = a (circular-conv) g,  g = Re(ifft(1 / fft(delta - w)))
         => out[r, n] = sum_m a[r, m] * H[m, n],  H[m, n] = g[(n - m) mod 63]

So the whole op is X[524288, 63] @ H[63, 63].  Data-parallel over 8 cores
(65536 rows each).  Per core (cost-model timeline ~99 us; DMA busy floor
~92 us, PE ~90 us — the ridge):

  DMA in  [128, 1008] f32 (4 KB/partition, contiguous HBM)       (sync/HWDGE)
  PE      transpose row-pairs  [128,126] -> [126,128] psum       (4 per bank)
  DVE     copy  psum_t [126,512] -> sbuf                         (1 per group)
  PE      matmul lhsT=Xt[126,128], rhs=blockdiag(H,H)[126,126]
          -> out [128,126] psum  == natural row-pair layout      (4 per bank)
  ACT     copy  psum_o -> back into the io SBUF tile (in place)  (1 per group)
  DMA out [128, 1008]                                            (scalar/HWDGE)

The io SBUF buffer is fully resident (32 slots, 129 KB/partition): loads
never reuse a slot, and outputs overwrite the consumed inputs in place, so
no DMA ever needs a slot-release wait.  This matters because this walrus
build rejects instructions with >2 sync commands (waits+incs): every
engine instruction here carries at most 1 wait.  Tiny "absorber" ops (din/
dxt/po-opener on PE, tr on ACT) each consume one cross-engine wait ahead
of the real work, `_strip_redundant_waits` removes provably-satisfied
same-engine waits, and `_patch_drain` splits the kernel-tail drain's 11
waits across a chain of drains.
"""

import numpy as np

SCOPE = 63
N_CORES = 8
ROWS_PER_CORE = 64 * 64 * 128 // N_CORES  # 65536
NL = 32                    # DMA loads per core
LOAD_F = 1008              # f32 per partition per load (8 rows of 63)
GROUPS_PER_LOAD = 2        # 504 f32 each
PAIR = 126                 # one double-row
TILES_PER_GROUP = 4        # 4 double-tiles = 1 psum bank

_CACHE = {}


def _inverse_kernel(inhibition_filter: np.ndarray) -> np.ndarray:
    """g = Re(ifft(1 / fft(delta - w))) computed in float64 on host."""
    k = -inhibition_filter.astype(np.float64)
    k[0] += 1.0
    return np.real(np.fft.ifft(1.0 / np.fft.fft(k)))


def _build_consts(inhibition_filter: np.ndarray):
    g = _inverse_kernel(inhibition_filter)
    idx = (np.arange(SCOPE)[None, :] - np.arange(SCOPE)[:, None]) % SCOPE
    H = g[idx].astype(np.float32)          # [m, n]
    H2 = np.zeros((2 * SCOPE, 2 * SCOPE), np.float32)
    H2[:SCOPE, :SCOPE] = H
    H2[SCOPE:, SCOPE:] = H
    ident = np.eye(128, dtype=np.float32)
    return H2, ident


def _strip_redundant_waits(nc):
    """Remove same-engine semaphore waits that are guaranteed satisfied.

    Engines execute their (non-DMA) instructions in order and increment
    their engine semaphore at completion.  A wait `sem_E >= v` on the k-th
    instruction of engine E is therefore always satisfied when v <= the
    cumulative increments from E's earlier non-DMA instructions.  Walrus
    rejects instructions with too many sync commands, and Tile emits these
    provably-redundant waits (its minimality is per-proc, not transitive).
    DMA completions are asynchronous; their sems are never inc'd by engine
    instruction retirement, so they are untouched.
    """
    insts = list(nc.all_instructions())
    # sem id -> set of (engine, is_dma) that update it
    updaters = {}
    for i in insts:
        eng = i.engine
        si = i.sync_info
        if si is None:
            continue
        is_dma = type(i).__name__ in ("InstDMACopy", "InstDmaTransposeAnt",
                                      "InstTensorLoad", "InstTensorSave",
                                      "InstCollectiveCompute")
        for u in (si.on_update or []):
            if u.update_mode == "sem-inc":
                updaters.setdefault(u.id, set()).add((eng, is_dma))

    def engine_private(sem_id, eng):
        ups = updaters.get(sem_id, set())
        return ups and all(e == eng and not d for (e, d) in ups)

    counts = {}
    removed = 0
    for i in insts:
        eng = i.engine
        si = i.sync_info
        if si is None:
            continue
        waits = si.on_wait or []
        if waits:
            keep = []
            for w in waits:
                if (w.wait_mode == "sem-ge-imm"
                        and engine_private(w.id, eng)
                        and w.wait_value <= counts.get((eng, w.id), 0)):
                    removed += 1
                else:
                    keep.append(w)
            if len(keep) != len(waits):
                si.on_wait = keep
        is_dma = type(i).__name__ in ("InstDMACopy", "InstDmaTransposeAnt",
                                      "InstTensorLoad", "InstTensorSave",
                                      "InstCollectiveCompute")
        if not is_dma:
            for u in (si.on_update or []):
                if u.update_mode == "sem-inc":
                    key = (eng, u.id)
                    counts[key] = counts.get(key, 0) + u.update_value
    return removed


def _patch_drain(tile_mod):
    """Split the kernel-tail drain's sem waits across a chain of drains.

    Tile emits one Drain instruction waiting on every proc (11 sems); this
    walrus build rejects instructions with that many sync commands.  SP
    executes the chain serially, so N drains with <=1 wait each are
    equivalent.
    """
    if getattr(tile_mod.TileContext, "_drain_split_patched", False):
        return
    from concourse.vector_clock import ScopedClock

    def _drain_and_barrier(self, tick_clock, wait_clock):
        nc = self.nc
        drain_inst = nc.sync.drain()
        wait_clock.add_sem_waits(
            drain_inst.ins, ScopedClock({None: tick_clock.global_clock})
        )
        si = drain_inst.ins.sync_info
        waits = list((si.on_wait if si else None) or [])
        if len(waits) > 1:
            si.on_wait = waits[:1]
            for w in waits[1:]:
                d2 = nc.sync.drain()
                si2 = d2.ins.sync_info
                if si2 is None:
                    wait_clock.add_sem_waits(
                        d2.ins, ScopedClock({None: tick_clock.global_clock})
                    )
                    si2 = d2.ins.sync_info
                si2.on_wait = [w]
        nc.all_engine_barrier()
        assert self.sems is not None
        popped = nc._tile_sem_poison_stack.pop()
        assert popped is self._sem_poison
        nc.clear_and_free_semaphores(list(self.sems.allocated().values()))
        nc.all_engine_barrier()

    tile_mod.TileContext._drain_and_barrier = _drain_and_barrier
    tile_mod.TileContext._drain_split_patched = True


def _build_program(nl: int, in_bufs: int = 3, st_bufs: int = 3,
                   split_load: int = 1, alt_load_engine: bool = False,
                   const_eng: str = "gpsimd", load_eng: str = "sync",
                   store_eng: str = "scalar", strip: bool = True,
                   pt_bufs: int = 3, po_bufs: int = 4, xt_bufs: int = 8,
                   warm_spin: int = 0, t_f32r: bool = False,
                   grp_store: bool = False):
    import concourse.bass as bass
    import concourse.mybir as mybir
    from concourse import tile

    _patch_drain(tile)

    f32 = mybir.dt.float32
    f32r = mybir.dt.float32r
    nc = bass.Bass("TRN2", target_bir_lowering=False, debug=False,
                   num_devices=N_CORES)

    act = nc.dram_tensor("activations", [nl, 128, LOAD_F], f32,
                         kind="ExternalInput").ap()
    h2d = nc.dram_tensor("h2", [PAIR, PAIR], f32, kind="ExternalInput").ap()
    idd = nc.dram_tensor("ident", [128, 128], f32, kind="ExternalInput").ap()
    out = nc.dram_tensor("out", [nl, 128, LOAD_F], f32,
                         kind="ExternalOutput").ap()

    with tile.TileContext(nc) as tc:
        with (
            tc.tile_pool(name="consts", bufs=1) as consts,
            tc.tile_pool(name="inp", bufs=nl) as inp,
            tc.tile_pool(name="xt", bufs=xt_bufs) as xtp,
            tc.tile_pool(name="trash", bufs=1) as trp,
            tc.tile_pool(name="psum_t", bufs=pt_bufs, space="PSUM") as pst,
            tc.tile_pool(name="psum_o", bufs=po_bufs, space="PSUM") as pso,
            tc.tile_pool(name="scratch", bufs=1, space="PSUM") as scratch,
        ):
            eng = {"sync": nc.sync, "scalar": nc.scalar, "gpsimd": nc.gpsimd}
            h2_sb = consts.tile([PAIR, PAIR], f32)
            id_sb = consts.tile([128, 128], f32)
            eng[const_eng].dma_start(out=h2_sb[:], in_=h2d)
            eng[const_eng].dma_start(out=id_sb[:], in_=idd)

            if warm_spin:
                # Spin the PE on a zeroed tile while the first load is in
                # flight: ramps the HAM clock gate / pstate before real work.
                zt = consts.tile([128, 128], f32, tag="warmz")
                nc.gpsimd.memset(zt[:], 0.0)
                for w in range(warm_spin):
                    ztp = scratch.tile([PAIR, 128], f32, tag="din")
                    nc.tensor.matmul(ztp[:], zt[:, 0:PAIR], zt[:],
                                     is_transpose=True)
            # Warm-up dummies: consume the const-load waits on PE once.
            wt = scratch.tile([PAIR, 128], f32, tag="din")
            nc.tensor.matmul(wt[:], id_sb[:, 0:PAIR], id_sb[:],
                             is_transpose=True)
            wo = scratch.tile([128, PAIR], f32, tag="din")
            nc.tensor.matmul(wo[:], id_sb[0:PAIR, :], h2_sb[:])

            for L in range(nl):
                # Full-residency i/o buffer: every load gets a fresh SBUF
                # slot (no WAW/release waits on the DMA) and the outputs are
                # written back in place, so the store only waits on copy2.
                in_t = inp.tile([128, LOAD_F], f32)
                ld_eng = nc.gpsimd if (alt_load_engine and L % 2) else eng[load_eng]
                ld_eng.dma_start(out=in_t[:], in_=act[L])
                # ACT wait-absorber: observe the load completion on ACT so
                # the in-place copy2 below only waits on PE.
                tr = trp.tile([1, 1], f32, tag="tr")
                nc.scalar.copy(tr[:], in_t[0:1, 0:1])
                for grp in range(GROUPS_PER_LOAD):
                    base = 504 * grp
                    # PE wait-absorber: a 1x1 matmul reading in_t consumes the
                    # load-DMA-done wait, so the first transpose below only
                    # waits on its psum slot release.
                    din = scratch.tile([1, 1], f32, tag="din")
                    nc.tensor.matmul(din[:], in_t[0:1, base:base + 1],
                                     id_sb[0:1, 0:1])
                    pt = pst.tile([PAIR, 512], f32)
                    for k in range(TILES_PER_GROUP):
                        off = base + PAIR * k
                        # Note: t_f32r stays False — the BIR verifier
                        # requires fp32r inputs pre-rounded (it is a reduced
                        # precision format), so it would truncate the data.
                        cast = (lambda ap: ap.bitcast(f32r)) if t_f32r else (lambda ap: ap)
                        nc.tensor.matmul(
                            cast(pt[:, 128 * k:128 * (k + 1)]),
                            cast(in_t[:, off:off + PAIR]),
                            cast(id_sb[:]),
                            is_transpose=True,
                            start=(k == 0),
                            stop=(k == TILES_PER_GROUP - 1),
                        )
                    xt = xtp.tile([PAIR, 512], f32)
                    nc.vector.tensor_copy(xt[:], pt[:])
                    po = pso.tile([128, 512], f32)
                    # PE wait-absorber for the psum_o slot release (ACT): the
                    # group opener writes spare column 504 (never read), so
                    # the real matmuls below carry no ACT wait.
                    nc.tensor.matmul(po[:, 504:505], id_sb[0:1, :],
                                     id_sb[0:1, 0:1], start=True, stop=False)
                    # PE wait-absorber: consumes the xt-ready (DVE) wait.
                    dxt = scratch.tile([1, 1], f32, tag="din")
                    nc.tensor.matmul(dxt[:], xt[0:1, 0:1], id_sb[0:1, 0:1])
                    for k in range(TILES_PER_GROUP):
                        nc.tensor.matmul(
                            po[:, PAIR * k:PAIR * (k + 1)],
                            xt[:, 128 * k:128 * (k + 1)],
                            h2_sb[:],
                            start=False,
                            stop=(k == TILES_PER_GROUP - 1),
                        )
                    nc.scalar.copy(in_t[:, base:base + 504], po[:, 0:504])
                    if grp_store:
                        eng[store_eng].dma_start(
                            out=out[L][:, base:base + 504],
                            in_=in_t[:, base:base + 504])
                if not grp_store:
                    eng[store_eng].dma_start(out=out[L], in_=in_t[:])

    if strip:
        # Safe on hardware (engines retire instructions in order); the
        # CoreSim race detector, however, only understands semaphores, so
        # sim runs should build with strip=False.
        _strip_redundant_waits(nc)
    return nc


def _get_program(nl: int):
    if nl not in _CACHE:
        _CACHE[nl] = _build_program(nl)
    return _CACHE[nl]


def kernel(activations: np.ndarray, inhibition_filter: np.ndarray,
           trace: bool = False):
    from concourse.bass_utils import run_bass_kernel_spmd

    activations = np.ascontiguousarray(activations, dtype=np.float32)
    H2, ident = _build_consts(np.asarray(inhibition_filter, np.float32))

    shards = activations.reshape(N_CORES, NL, 128, LOAD_F)
    in_maps = [
        {"activations": shards[c], "h2": H2, "ident": ident}
        for c in range(N_CORES)
    ]
    nc = _get_program(NL)
    try:
        res = run_bass_kernel_spmd(nc, in_maps, list(range(N_CORES)),
                                   trace=trace)
    except ModuleNotFoundError:
        # NTFF profile hook unavailable in this axon build
        res = run_bass_kernel_spmd(nc, in_maps, list(range(N_CORES)),
                                   trace=False)
    full = np.concatenate([res.results[c]["out"].reshape(1, -1)
                           for c in range(N_CORES)], axis=0)
    out = full.reshape(64, 64, 128, SCOPE)
    if trace:
        kernel.last_results = res
    return out
